# revision 56
# baseline (speedup 1.0000x reference)
"""Causal single-head attention on 8 Trainium2 NeuronCores — fully local.

Problem: x[4096,1024] -> Q,K,V = x@W.T+b (d_k=64), out = softmax(causal(QK^T/8)) @ V.

Strategy (replicated K/V, packed stream, zero communication) — v4:
  - Every core computes K^T/V^T for all 32 key blocks locally; no collective.
  - Core c owns query blocks {c, 8+c, 16+c, 24+c} (strided); identical SPMD
    program; core's own 512 x columns ride a dedicated xq input; the main x
    stream is host-packed with those columns deleted (7MB instead of 8MB).
  - Bias algebra: bk is dropped entirely (softmax over keys is invariant to
    a per-query constant, and Q.bk is constant across keys); bv is folded
    into the V^T PSUM->SBUF cast as a per-partition scalar add (attn rows
    sum to 1, so V+bv passes through exactly); bq stays on the Q path.
  - K^T (rows 0:64) and V^T (rows 64:128) share one PSUM matmul chain and
    ONE Pool-engine tensor_scalar_add per chunk (cast f32->bf16 + bv).
  - V~ blocks ([key, dk] layout) via PE transpose of V^T (bf16) + DVE copy;
    ones column memset once (denominator accumulator).
  - Attention per (slot, group): scores [key, q] -> exp on Act (scale=1/8,
    PSUM->SBUF bf16) -> band mask via per-block scalar columns (rank-encoded
    0/1 in a tiny f32 input) on DVE; triangular mask on the own-diagonal.
  - AV is FLIPPED vs v3: lhsT = e (stationary), rhs = V~ (moving) so the
    out free dim is 65 (dk+1) instead of 128: avp[j] = [q=128, dk+1] in a
    per-slot PERSISTENT psum bank accumulated as one open group across all
    of slot j's key blocks (own first, then packed groups in order).
    Epilogue needs NO transpose: reciprocal of col 64 + tensor_scalar_mul.
  - PSUM banks: 2 kv chains + 2 scores/scratch + 4 per-slot avp = 8.
  - DMA stream order: constants -> xq (2x256) -> packed chunks
    (6x512 + 256 + 128 + 128) so compute starts early and the tail is thin.
"""

import os
import numpy as np
import ml_dtypes
from contextlib import ExitStack

S, DM, DK = 4096, 1024, 64
NCORES = 8
QB = 128                      # rows per block
SLOTS = 4                     # q-blocks per core
SH = QB * SLOTS               # 512 own query rows per core
NB = S // QB                  # 32 key blocks
SP = S - SH                   # 3584 packed (non-own) x columns
NPB = SP // QB                # 28 packed key blocks
ND = DM // 128                # 8 contraction chunks

# cw bf16 blob: wkv [8, 128] | wq [8, 64] | identcol [64] | tri [128]
BW_WKV = 0
BW_WQ = ND * 128
BW_ID = BW_WQ + ND * DK
BW_TRI = BW_ID + 64
BW_COLS = BW_TRI + QB
# cf f32 blob: col0 = [0*64, bv] | col1 = [bq, 0*64] | cols 2..8 = band bits
CF_COLS = 9

AMP = int(os.environ.get("KERNEL_AMP", "1"))  # repeat whole pipeline in-NEFF

LAST_EXEC_NS = None

# packed x chunk schedule: 512-col chunks (fewer cross-engine coupling
# cycles), thin 256-col tail for a short end chain
CHUNKS = [(0, 512), (512, 1024), (1024, 1536), (1536, 2048), (2048, 2560),
          (2560, 3072), (3072, 3328), (3328, 3584)]


def _build_nc():
    import concourse.bass as bass
    import concourse.bacc as bacc
    import concourse.mybir as mybir
    import concourse.tile as tile

    f32 = mybir.dt.float32
    bf16 = mybir.dt.bfloat16
    AF = mybir.ActivationFunctionType

    nc = bacc.Bacc(None, num_devices=NCORES)

    xT_d = nc.dram_tensor("xT", [DM, SP], bf16, kind="ExternalInput")
    xqT_d = nc.dram_tensor("xqT", [DM, SH], bf16, kind="ExternalInput")
    cw_d = nc.dram_tensor("cw", [128, BW_COLS], bf16, kind="ExternalInput")
    cf_d = nc.dram_tensor("cf", [128, CF_COLS], f32, kind="ExternalInput")
    out_d = nc.dram_tensor("out", [SH, DK], bf16, kind="ExternalOutput")

    with tile.TileContext(nc) as tc, ExitStack() as ctx:
        singles = ctx.enter_context(tc.tile_pool(name="singles", bufs=1))
        kvps = ctx.enter_context(tc.tile_pool(name="kvps", bufs=2,
                                              space="PSUM"))
        scps = ctx.enter_context(tc.tile_pool(name="scps", bufs=4,
                                              space="PSUM"))
        perps = ctx.enter_context(tc.tile_pool(name="perps", bufs=1,
                                               space="PSUM"))
        epool = ctx.enter_context(tc.tile_pool(name="epool", bufs=4))

        cw_sb = singles.tile([128, BW_COLS], bf16)
        cf_sb = singles.tile([128, CF_COLS], f32)
        wkv_sb = cw_sb[:, BW_WKV:BW_WQ].rearrange("p (d c) -> p d c", d=ND)
        wq_sb = cw_sb[:, BW_WQ:BW_ID].rearrange("p (d c) -> p d c", d=ND)
        identcol_sb = cw_sb[:, BW_ID:BW_ID + 64]
        tri_sb = cw_sb[:, BW_TRI:BW_TRI + QB]
        bkv_sb = cf_sb[:, 0:1]
        bq_sb = cf_sb[0:DK, 1:2]

        xT_sb = singles.tile([128, ND, SP], bf16)
        xq_sb = singles.tile([128, ND, SH], bf16)
        # K^T rows 0:64, V^T rows 64:128; packed cols then own cols
        kv_sb = singles.tile([128, SP + SH], bf16)
        vt_sb = singles.tile([128, NB, DK + 1], bf16)   # packed | own
        # band-masked copies of packed V~ blocks (mask folded off-path):
        # packed position p sits in the band of slot p//7 at offset p%7
        vtb_sb = singles.tile([128, NPB, DK + 1], bf16)
        qT_sb = singles.tile([DK, SH], bf16)

        def one_pass(rep):
            # ---------- DMA issue order == DMA_ENGINES service order -------
            # wkv weights -> first packed chunks (PE starts early) -> xq/own
            # -> remaining constants -> rest of the packed stream.
            def load_chunk(c0, c1):
                nc.sync.dma_start(
                    out=xT_sb[:, :, c0:c1],
                    in_=xT_d[:, c0:c1].rearrange("(d p) s -> p d s", p=128))

            nc.sync.dma_start(out=cw_sb, in_=cw_d[:, :])
            nc.sync.dma_start(out=cf_sb, in_=cf_d[:, :])
            nc.sync.dma_start(
                out=xq_sb[:, :, 0:256],
                in_=xqT_d[:, 0:256].rearrange("(d p) s -> p d s", p=128))
            nc.sync.dma_start(
                out=xq_sb[:, :, 256:512],
                in_=xqT_d[:, 256:512].rearrange("(d p) s -> p d s", p=128))
            for c0, c1 in CHUNKS:
                load_chunk(c0, c1)

            # ones column of V~ (denominator accumulator)
            nc.gpsimd.memset(vt_sb[:, :, DK:DK + 1], 1.0)

            # PE p-state warmup: a few dependency-free matmuls at t~0 start
            # the tensor-engine ramp clock while the first DMAs stream in
            warm_sb = singles.tile([128, 128], bf16, name="warm")
            nc.gpsimd.memset(warm_sb, 1.0)
            warm_ps = kvps.tile([128, 2, 256], f32, tag="kvps", bufs=2,
                                name="warm_ps")
            for w in range(46):
                nc.tensor.matmul(warm_ps[:, w % 2, 0:128], lhsT=warm_sb,
                                 rhs=warm_sb, start=True, stop=True)

            avp_pair = [perps.tile([QB, 2, DK + 1], f32, name=f"avpp{p}")
                        for p in range(2)]
            pair_started = [False, False]
            # AV matmuls left per slot (own j+1 + packed 7j+7)
            mm_left = [(j + 1) + (7 * j + 7) for j in range(SLOTS)]
            pair_left = [mm_left[0] + mm_left[1], mm_left[2] + mm_left[3]]

            def emit_proj(c0, W, xsrc, kbase):
                # project x cols [c0, c0+W) into kv_sb/vt_sb at col base
                # kbase + c0 (kbase=0 packed, kbase=SP own)
                h = W // 2
                cs0 = slice(c0, c0 + h)
                cs1 = slice(c0 + h, c0 + W)
                kv_ps = kvps.tile([128, 2, 256], f32, tag="kvps", bufs=2,
                                  name="kv_ps")
                kv0, kv1 = kv_ps[:, 0, 0:h], kv_ps[:, 1, 0:h]
                for d in range(ND):
                    nc.tensor.matmul(kv0, lhsT=wkv_sb[:, d, :],
                                     rhs=xsrc[:, d, cs0],
                                     start=(d == 0), stop=False,
                                     skip_group_check=True)
                    nc.tensor.matmul(kv1, lhsT=wkv_sb[:, d, :],
                                     rhs=xsrc[:, d, cs1],
                                     start=False, stop=(d == ND - 1),
                                     skip_group_check=True)
                kc0 = kbase + c0
                # cast f32->bf16 + bv on V rows (bk dropped); DVE:
                # GPSIMD cannot access PSUM (BIR verifier)
                nc.vector.tensor_scalar_add(
                    kv_sb[:, kc0:kc0 + W].rearrange("k (h s) -> k h s", h=2),
                    kv_ps[:, :, 0:h], bkv_sb)

                # V~ chain (PE transpose + DVE copy + Pool band-mask) is
                # DEFERRED one chunk: the transposes read the cast's output
                # and would otherwise park at the head of PE's in-order
                # queue, stalling the whole engine.
                def vt_chain(kc0=kc0, W=W, kbase=kbase):
                    nblk = W // QB
                    t_ps = kvps.tile([128, 2, 512], bf16, tag="kvps",
                                     bufs=2, name="t_ps")
                    for sl in range(nblk):
                        nc.tensor.transpose(
                            t_ps[:, sl, 0:DK],
                            kv_sb[64:128, kc0 + QB * sl:kc0 + QB * (sl + 1)],
                            identcol_sb[64:128, :])
                    kb0 = kc0 // QB
                    nc.vector.tensor_copy(vt_sb[:, kb0:kb0 + nblk, 0:DK],
                                          t_ps[:, 0:nblk, 0:DK])
                    if kbase == 0:
                        for kb in range(kb0, kb0 + nblk):
                            mcol = cf_sb[:, 2 + kb % 7:3 + kb % 7]
                            nc.gpsimd.tensor_scalar_mul(
                                vtb_sb[:, kb, :], vt_sb[:, kb, :], mcol)
                vt_pending.append(vt_chain)

            def emit_q(h0, h1):
                q_t = kvps.tile([128, 2, 256], f32, tag="kvps", bufs=2,
                                name="q_t")
                q_ps = q_t[0:DK, :, :].rearrange("p a b -> p (a b)")
                for d in range(ND):
                    nc.tensor.matmul(q_ps[:, h0:h1], lhsT=wq_sb[:, d, :],
                                     rhs=xq_sb[:, d, h0:h1],
                                     start=(d == 0), stop=(d == ND - 1))
                nc.vector.tensor_scalar_add(qT_sb[:, h0:h1],
                                            q_ps[:, h0:h1], bq_sb)

            def emit_epilogue(j, src_ap):
                rec = epool.tile([QB, 1], f32, tag="rec", name="rec")
                nc.vector.reciprocal(rec, src_ap[:, DK:DK + 1])
                out_sb = epool.tile([QB, DK], bf16, tag="osb", name="out_sb")
                nc.vector.tensor_scalar_mul(out_sb, src_ap[:, 0:DK], rec)
                nc.sync.dma_start(out=out_d[QB * j:QB * (j + 1), :],
                                  in_=out_sb)

            def emit_av_group(j, e_sb, pairs):
                # two slots share one persistent psum bank (column-split
                # chains, single open group, per-element has_written); stop
                # fires on the bank's final matmul across both slots
                p = j // 2
                n = len(pairs)
                mm_left[j] -= n
                pair_left[p] -= n
                avp = avp_pair[p][:, j % 2, :]
                for i, (idx, vt_ap) in enumerate(pairs):
                    nc.tensor.matmul(avp, lhsT=e_sb[:, idx, :], rhs=vt_ap,
                                     start=(not pair_started[p]),
                                     stop=(pair_left[p] == 0 and i == n - 1),
                                     skip_group_check=True)
                    pair_started[p] = True
                if mm_left[j] == 0:
                    emit_epilogue(j, avp)

            pending = []   # deferred AV closures (exp given time to finish)
            vt_pending = []   # deferred V~ chains (cast given time to land)

            def flush_vt(keep=0):
                while len(vt_pending) > keep:
                    vt_pending.pop(0)()

            def flush():
                while pending:
                    pending.pop(0)()

            def emit_scores(j, blocks, own=False):
                # scores + exp for one group; AV deferred via `pending`.
                # blocks: list of ("o", own_idx) / ("p", packed_pos)
                nb_ = len(blocks)
                sc_ps = scps.tile([128, 4, 128], f32, tag="scps", bufs=4,
                                  name="sc_ps")
                e_sb = epool.tile([128, 4, QB], bf16, tag="e", bufs=8,
                                  name="e_sb")
                for sl, (t, kb) in enumerate(blocks):
                    col = (SP + QB * kb) if t == "o" else QB * kb
                    nc.tensor.matmul(
                        sc_ps[:, sl, :], lhsT=kv_sb[0:DK, col:col + QB],
                        rhs=qT_sb[:, QB * j:QB * (j + 1)],
                        start=True, stop=True)
                nc.scalar.activation(e_sb[:, 0:nb_, :], sc_ps[:, 0:nb_, :],
                                     AF.Exp, scale=0.125)
                band0 = 7 * j
                pairs = []
                for sl, (t, kb) in enumerate(blocks):
                    if t == "o":
                        if kb == j:
                            # triangular mask on the diagonal block
                            nc.gpsimd.tensor_tensor(
                                e_sb[:, sl, :], e_sb[:, sl, :], tri_sb,
                                op=mybir.AluOpType.mult)
                        pairs.append((sl, vt_sb[:, NPB + kb, :]))
                    else:
                        src_vt = vtb_sb if kb >= band0 else vt_sb
                        pairs.append((sl, src_vt[:, kb, :]))
                pending.append(lambda: emit_av_group(j, e_sb, pairs))

            def attn_for_ready(lo, hi):
                for j in range(SLOTS):
                    pref = 7 * j + 7
                    for g0 in range(0, pref, 4):
                        nb_ = min(4, pref - g0)
                        if lo < g0 + nb_ <= hi:
                            emit_scores(j, list(range(g0, g0 + nb_)), False)

            readyq = []

            def pump_av(force=False):
                while pending and (force or len(pending) > 4):
                    pending.pop(0)()

            def drain_scores(cap):
                n = 0
                while readyq and n < cap:
                    j, blocks, own = readyq.pop(0)
                    emit_scores(j, blocks, own)
                    pump_av()
                    n += 1

            # unified per-slot block sequence: own blocks (as ("o", i))
            # then packed positions (("p", p)), chunked into 4-block groups
            GROUPS = []
            for j in range(SLOTS):
                seq = [("o", i) for i in range(j + 1)] + \
                      [("p", p) for p in range(7 * j + 7)]
                ends = list(range(4, len(seq) + 1, 4))
                if len(seq) % 4:
                    ends.append(len(seq))
                if j == 3 and len(ends) >= 2:
                    ends = ends[:-1] + [len(seq) - 2, len(seq)]  # thin tail
                g0 = 0
                for e in ends:
                    GROUPS.append((j, seq[g0:e]))
                    g0 = e

            def max_packed(blocks):
                ps = [p for t, p in blocks if t == "p"]
                return max(ps) + 1 if ps else 0

            def add_ready(lo, hi):
                # a group is ready once its highest packed position is
                # castable; all-own groups have threshold 0
                for j, blocks in GROUPS:
                    if lo < max_packed(blocks) <= hi:
                        readyq.append((j, blocks, False))

            # ---------------- compute emission --------------------------
            # software pipeline: scores run a chunk behind the cast; AV
            # groups drip out >=2 groups behind their exp; epilogues fire
            # from the AV closures.
            ck = list(CHUNKS)
            emit_proj(0, 256, xq_sb, SP)       # own cols 0:256 (blocks 0,1)
            emit_q(0, 256)
            emit_proj(256, 256, xq_sb, SP)     # own cols 256:512
            emit_q(256, 512)
            emit_proj(ck[0][0], 256, xT_sb, 0)
            flush_vt()          # own-chunk V~ chains
            add_ready(-1, 0)    # all-own groups
            drain_scores(2)
            emit_proj(ck[1][0], 256, xT_sb, 0)
            flush_vt()          # chunk-0 V~ chain
            add_ready(0, 2)
            drain_scores(2)
            pump_av()
            avail = 2
            for k in range(2, len(ck)):
                emit_proj(ck[k][0], 256, xT_sb, 0)
                flush_vt()      # chunk k-1's V~ chain
                pump_av()
                lo, avail = avail, 2 * k
                add_ready(lo, avail)
                drain_scores(2)
            flush_vt()
            add_ready(avail, NPB)
            drain_scores(10 ** 9)
            pump_av(force=True)

        for _rep in range(AMP):
            one_pass(_rep)

    nc.finalize()
    return nc


def _in_maps(x, Wq, bq, Wk, bk, Wv, bv):
    bf = ml_dtypes.bfloat16
    xT = np.ascontiguousarray(x.T).astype(bf)              # [1024, 4096]
    tri = np.triu(np.ones((QB, QB), dtype=np.float32))     # e[k,q]: k<=q

    wkvT = np.concatenate([Wk.T, Wv.T], axis=1)            # [1024, 128]
    wkv_p = wkvT.reshape(ND, 128, 2 * DK).transpose(1, 0, 2).reshape(128, -1)
    wq_p = Wq.T.reshape(ND, 128, DK).transpose(1, 0, 2).reshape(128, -1)
    identcol = np.tile(np.eye(64, dtype=np.float32), (2, 1))  # [128, 64]
    cw = np.ascontiguousarray(np.concatenate(
        [wkv_p, wq_p, identcol, tri], axis=1).astype(bf))
    assert cw.shape == (128, BW_COLS)

    maps = []
    for c in range(NCORES):
        own = [8 * sl + c for sl in range(SLOTS)]
        rows = np.concatenate([np.arange(QB * b, QB * (b + 1)) for b in own])
        xqT = np.ascontiguousarray(x[rows].T).astype(bf)   # [1024, 512]
        keep = np.setdiff1d(np.arange(S), rows)
        xTp = np.ascontiguousarray(xT[:, keep])            # [1024, 3584]
        cf = np.zeros((128, CF_COLS), dtype=np.float32)
        cf[:, 0] = np.concatenate([np.zeros(DK, np.float32), bv])
        cf[0:DK, 1] = bq
        for k in range(7):
            cf[:, 2 + k] = 1.0 if k < c else 0.0
        maps.append({"xT": xTp, "xqT": xqT, "cw": cw, "cf": cf})
    return maps


def kernel(**inputs):
    global LAST_EXEC_NS
    x = np.asarray(inputs["x"], dtype=np.float32)
    args = [np.asarray(inputs[k], dtype=np.float32)
            for k in ("Wq", "bq", "Wk", "bk", "Wv", "bv")]
    in_maps = _in_maps(x, args[0], args[1], args[2], args[3], args[4], args[5])

    nc = _build_nc()
    from concourse.bass_utils import run_bass_kernel_spmd
    res = run_bass_kernel_spmd(nc, in_maps, core_ids=list(range(NCORES)))
    LAST_EXEC_NS = res.exec_time_ns

    out = np.zeros((S, DK), dtype=np.float32)
    for c in range(NCORES):
        r = np.asarray(res.results[c]["out"], dtype=np.float32)
        for sl in range(SLOTS):
            b = 8 * sl + c
            out[QB * b:QB * (b + 1)] = r[QB * sl:QB * (sl + 1)]
    return out
